# revision 20
# baseline (speedup 1.0000x reference)
"""Trainium2 Bass kernel for the buggy-softmax attention layer.

Shapes: hidden_states [4, 2048, 1024], Wq/Wk/Wv [1024, 1024], biases [1024].
reference: q,k,v proj -> per-head scores/sqrt(64) -> exp(s - max) / sum(RAW s)
-> @ v -> merge heads.

Sharding: 8 cores = 4 batches x 2 head-groups (8 heads of 64 dims each).
Each core gets x[b].T plus the head-group slice of the transposed weights and
produces out[b][:, hg*512:(hg+1)*512] in natural [seq, feat] layout.

Per-core algorithm (dual-orientation scores):
  qT = (x@Wq.T)/8 + bq/8   [512f, S]  fp32  (1/8 folded into the PSUM evac)
  kT = x@Wk.T + bk         [512f, S]  f32r  (rounded copy for fast matmuls)
  v  = x@Wv.T + bv         [S, 512f]  f32r  (bias via K=1 ones-matmul)
  den (sum of RAW scores) stays fp32-faithful: 128-key chunk sums of k
  (fp32, from the k PSUM) -> den = matmul(qT_h tile, ksum chunks) + DVE sum.
  m = rowmax(s) via f32r scores in [q,k] orientation + DVE reduce_max.
  numerator: sT = kT-tile^T contraction against qT -> exp(sT) -> E0T (sbuf);
  outT[64, q] += v-tile^T-contract @ E0T (f32r).
  out = transpose(outT) * exp(-m)/den (per-partition scale), DMA out.
"""

import numpy as np

B, S, D, H = 4, 2048, 1024, 16
HD = 64
NCORES = 8
HPC = 8            # heads per core
DG = HPC * HD      # 512 features per core

_CACHE = {}


def build(nc_S=S, dbg=False):
    import concourse.mybir as mybir
    import concourse.tile as tile
    from concourse import bacc
    from concourse.masks import make_identity

    f32 = mybir.dt.float32
    f32r = mybir.dt.float32r
    f16 = mybir.dt.float16
    FT = mybir.ActivationFunctionType
    ALU = mybir.AluOpType
    AX = mybir.AxisListType

    S_ = nc_S
    KP = D // 128            # 8 contraction chunks for projections
    PT = DG // 128           # 4 feature partition-tiles
    NW = min(512, S_)        # seq chunk width for projections / smax
    N5 = S_ // NW            # chunks per row
    ST = S_ // 128           # seq/key tiles of 128
    KC = S_ // 128           # 128-key ksum chunks
    QGW = min(512, S_)       # query-group width
    NQG = S_ // QGW
    QT_PER_G = QGW // 128

    nc = bacc.Bacc(None, target_bir_lowering=False, debug=dbg)

    xt_d = nc.declare_dram_parameter("xt", [D, S_], f32, isOutput=False)
    wqt_d = nc.declare_dram_parameter("wqt", [D, DG], f32, isOutput=False)
    wkt_d = nc.declare_dram_parameter("wkt", [D, DG], f32, isOutput=False)
    wvt_d = nc.declare_dram_parameter("wvt", [D, DG], f32, isOutput=False)
    bq8_d = nc.declare_dram_parameter("bq8", [128, PT], f32, isOutput=False)
    bk4_d = nc.declare_dram_parameter("bk4", [128, PT], f32, isOutput=False)
    bv_d = nc.declare_dram_parameter("bvrow", [1, DG], f32, isOutput=False)
    out_d = nc.declare_dram_parameter("out", [S_, DG], f32, isOutput=True)

    with tile.TileContext(nc) as tc:
        with (
            tc.tile_pool(name="const", bufs=1) as cpool,
            tc.tile_pool(name="qt", bufs=PT) as qt_pool,
            tc.tile_pool(name="ktr", bufs=PT) as ktr_pool,
            tc.tile_pool(name="vv", bufs=ST) as v_pool,
            tc.tile_pool(name="ksumc", bufs=PT) as ks_pool,
        ):
            ident = cpool.tile([64, 64], f32, tag="ident")
            make_identity(nc, ident[:])
            identB_t = cpool.tile([128, 64], f32, tag="identB")
            make_identity(nc, identB_t[64:128, :])
            identB = identB_t[64:128, :]
            bq8 = cpool.tile([128, PT], f32, tag="bq8")
            bk4 = cpool.tile([128, PT], f32, tag="bk4")
            bvb = cpool.tile([128, DG], f32, tag="bvb")
            nc.sync.dma_start(bq8[:], bq8_d[:])
            nc.sync.dma_start(bk4[:], bk4_d[:])
            nc.sync.dma_start(bvb[:], bv_d[:].to_broadcast([128, DG]))

            qt_sb = [qt_pool.tile([128, S_], f32, tag="qt", name="qt_sb") for _ in range(PT)]
            ktr_sb = [ktr_pool.tile([128, S_], f16, tag="ktr", name="ktr_sb") for _ in range(PT)]
            v_sb = [v_pool.tile([128, DG], f16, tag="vv", name="v_sb") for _ in range(ST)]
            ksumc = [ks_pool.tile([128, KC], f32, tag="ks", name="ksumc") for _ in range(PT)]

            # ---------------- Phase 1: projections ----------------
            with (
                tc.tile_pool(name="xt", bufs=KP) as xt_pool,
                tc.tile_pool(name="w", bufs=10) as w_pool,
                tc.tile_pool(name="xtr", bufs=3) as xtr_pool,
                tc.tile_pool(name="kss", bufs=2) as kss_pool,
                tc.tile_pool(name="p1", bufs=4, space="PSUM") as p1_pool,
            ):
                xt_sb = [xt_pool.tile([128, S_], f32, tag="xt", name="xt_sb")
                         for _ in range(KP)]
                for k8 in range(KP):
                    nc.sync.dma_start(xt_sb[k8][:],
                                      xt_d[k8 * 128:(k8 + 1) * 128, :])

                # --- q projection (fp32; 1/8 + bias folded into evac)
                wq = [w_pool.tile([128, DG], f32, tag="w", name="w_t") for _ in range(KP)]
                for k8 in range(KP):
                    nc.sync.dma_start(wq[k8][:],
                                      wqt_d[k8 * 128:(k8 + 1) * 128, :])
                for p in range(PT):
                    for n in range(N5):
                        ps = p1_pool.tile([128, NW], f32, tag="p1")
                        for k8 in range(KP):
                            nc.tensor.matmul(
                                ps[:], wq[k8][:, p * 128:(p + 1) * 128],
                                xt_sb[k8][:, n * NW:(n + 1) * NW],
                                start=(k8 == 0), stop=(k8 == KP - 1))
                        nc.scalar.activation(
                            qt_sb[p][:, n * NW:(n + 1) * NW], ps[:],
                            FT.Identity, bias=bq8[:, p:p + 1], scale=0.125)

                # --- k projection (fp32; f32r evac + fp32 128-key chunk sums)
                wk = [w_pool.tile([128, DG], f32, tag="w", name="w_t") for _ in range(KP)]
                for k8 in range(KP):
                    nc.sync.dma_start(wk[k8][:],
                                      wkt_d[k8 * 128:(k8 + 1) * 128, :])
                for p in range(PT):
                    for n in range(N5):
                        ps = p1_pool.tile([128, NW], f32, tag="p1")
                        for k8 in range(KP):
                            nc.tensor.matmul(
                                ps[:], wk[k8][:, p * 128:(p + 1) * 128],
                                xt_sb[k8][:, n * NW:(n + 1) * NW],
                                start=(k8 == 0), stop=(k8 == KP - 1))
                        nc.scalar.activation(
                            ktr_sb[p][:, n * NW:(n + 1) * NW], ps[:],
                            FT.Identity, bias=bk4[:, p:p + 1], scale=1.0)
                        nsub = NW // 128
                        for c in range(nsub):
                            ks_scr = kss_pool.tile([128, 128], f32, tag="kss")
                            col = n * nsub + c
                            nc.vector.tensor_scalar(
                                ks_scr[:], ps[:, c * 128:(c + 1) * 128],
                                bk4[:, p:p + 1], None, ALU.add,
                                op1=ALU.add,
                                accum_out=ksumc[p][:, col:col + 1])

                # --- v projection (f32r; bias via K=1 ones matmul)
                wv = [w_pool.tile([128, DG], f16, tag="w", name="wv_t") for _ in range(KP)]
                for k8 in range(KP):
                    nc.gpsimd.dma_start(wv[k8][:],
                                        wvt_d[k8 * 128:(k8 + 1) * 128, :])
                for st in range(ST):
                    ps = p1_pool.tile([128, DG], f32, tag="p1")
                    for k8 in range(KP):
                        xtr = xtr_pool.tile([128, 128], f16, tag="xtr")
                        nc.vector.tensor_copy(
                            xtr[:], xt_sb[k8][:, st * 128:(st + 1) * 128])
                        nc.tensor.matmul(ps[:], xtr[:], wv[k8][:],
                                         start=(k8 == 0), stop=(k8 == KP - 1))
                    nc.vector.tensor_tensor(
                        v_sb[st][:], ps[:], bvb[:], op=ALU.add)

            # ---------------- Phase 2: attention ----------------
            with (
                tc.tile_pool(name="qtr", bufs=2) as qtr_pool,
                tc.tile_pool(name="e0t", bufs=17) as e_pool,
                tc.tile_pool(name="outT", bufs=5) as ot_pool,
                tc.tile_pool(name="ofin", bufs=4) as of_pool,
                tc.tile_pool(name="stats", bufs=2) as stat_pool,
                tc.tile_pool(name="pst", bufs=3, space="PSUM") as pst_pool,
                tc.tile_pool(name="psmax", bufs=2, space="PSUM") as psx_pool,
                tc.tile_pool(name="psav", bufs=1, space="PSUM") as pav_pool,
            ):
                for hp in range(HPC // 2):
                    h0, h1 = 2 * hp, 2 * hp + 1
                    qt_p = qt_sb[hp]                      # [128,S] fp32
                    kt_p = ktr_sb[hp]                     # [128,S] f32r
                    ks_p = ksumc[hp]                      # [128,KC] fp32

                    qtr_t = qtr_pool.tile([128, S_], f16, tag="qtr",
                                          name="qtr_t")
                    nc.vector.tensor_copy(qtr_t[:], qt_p[:])

                    for qg in range(NQG):
                        qsl = slice(qg * QGW, (qg + 1) * QGW)
                        # max + den per query tile (row-packed head pairs)
                        m2 = stat_pool.tile([128, 8 * QT_PER_G], f32,
                                            tag="m2", name="m2")
                        den_g = stat_pool.tile([128, 2 * QT_PER_G], f32,
                                               tag="den", name="den_g")
                        KHW = min(1024, S_)
                        NKH = S_ // KHW
                        for j in range(QT_PER_G):
                            qt_i = qg * QT_PER_G + j
                            qq = slice(qt_i * 128, (qt_i + 1) * 128)
                            for kh in range(NKH):
                                psxA = psx_pool.tile([128, KHW], f32,
                                                     tag="smax", name="psxA")
                                for kc in range(KHW // NW):
                                    c0 = kh * KHW + kc * NW
                                    nc.tensor.matmul(
                                        psxA[:, kc * NW:(kc + 1) * NW],
                                        qtr_t[0:64, qq],
                                        kt_p[0:64, c0:c0 + NW],
                                        start=True, stop=True)
                                psxB = psx_pool.tile([128, KHW], f32,
                                                     tag="smax", name="psxB")
                                for kc in range(KHW // NW):
                                    c0 = kh * KHW + kc * NW
                                    nc.tensor.matmul(
                                        psxB[:, kc * NW:(kc + 1) * NW],
                                        qtr_t[64:128, qq],
                                        kt_p[64:128, c0:c0 + NW],
                                        start=True, stop=True)
                                nc.vector.reduce_max(
                                    m2[:, 8 * j + kh:8 * j + kh + 1],
                                    psxA[:], axis=AX.X)
                                nc.vector.reduce_max(
                                    m2[:, 8 * j + 4 + kh:8 * j + 4 + kh + 1],
                                    psxB[:], axis=AX.X)
                            ps_dA = pst_pool.tile([128, KC], f32, tag="sT",
                                                  name="ps_dA")
                            nc.tensor.matmul(ps_dA[:], qt_p[0:64, qq],
                                             ks_p[0:64, :],
                                             start=True, stop=True)
                            nc.vector.reduce_sum(
                                den_g[:, 2 * j:2 * j + 1],
                                ps_dA[:], axis=AX.X)
                            ps_dB = pst_pool.tile([128, KC], f32, tag="sT",
                                                  name="ps_dB")
                            nc.tensor.matmul(ps_dB[:], qt_p[64:128, qq],
                                             ks_p[64:128, :],
                                             start=True, stop=True)
                            nc.vector.reduce_sum(
                                den_g[:, 2 * j + 1:2 * j + 2],
                                ps_dB[:], axis=AX.X)

                        # numerator: paired sT tiles -> exp -> E0T; then AV
                        e0, e1 = [], []
                        for kt in range(ST):
                            ksl = slice(kt * 128, (kt + 1) * 128)
                            psA = pst_pool.tile([128, QGW], f32, tag="sT",
                                                name="psA")
                            nc.tensor.matmul(psA[:], kt_p[0:64, ksl],
                                             qtr_t[0:64, qsl],
                                             start=True, stop=True)
                            psB = pst_pool.tile([128, QGW], f32, tag="sT",
                                                name="psB")
                            nc.tensor.matmul(psB[:], kt_p[64:128, ksl],
                                             qtr_t[64:128, qsl],
                                             start=True, stop=True)
                            etA = e_pool.tile([128, QGW], f16, tag="e0t",
                                              name="etA")
                            nc.scalar.activation(etA[:], psA[:], FT.Exp)
                            etB = e_pool.tile([128, QGW], f16, tag="e0t",
                                              name="etB")
                            nc.scalar.activation(etB[:], psB[:], FT.Exp)
                            e0.append(etA)
                            e1.append(etB)
                        ps_av = pav_pool.tile([64, QGW], f32, tag="av",
                                              name="ps_av")
                        for kt in range(ST):
                            nc.tensor.matmul(
                                ps_av[:], v_sb[kt][:, h0 * 64:h0 * 64 + 64],
                                e0[kt][:],
                                start=(kt == 0), stop=(kt == ST - 1))
                        ps_av2 = pav_pool.tile([64, QGW], f32, tag="av",
                                               name="ps_av2")
                        for kt in range(ST):
                            nc.tensor.matmul(
                                ps_av2[:], v_sb[kt][:, h1 * 64:h1 * 64 + 64],
                                e1[kt][:],
                                start=(kt == 0), stop=(kt == ST - 1))
                        oT = ot_pool.tile([128, QGW], f32, tag="outT",
                                          name="oT")
                        nc.scalar.copy(oT[0:64, :], ps_av[:])
                        nc.scalar.copy(oT[64:128, :], ps_av2[:])

                        # per-qg stats: scale = exp(-m) / den
                        m_g = stat_pool.tile([128, 2 * QT_PER_G], f32,
                                             tag="m", name="m_g")
                        nc.vector.reduce_max(
                            m_g[:],
                            m2[:].rearrange("p (q x) -> p q x",
                                            x=4)[:, :, 0:NKH],
                            axis=AX.X)
                        eneg = stat_pool.tile([128, 2 * QT_PER_G], f32,
                                              tag="eneg", name="eneg")
                        nc.scalar.activation(eneg[:], m_g[:], FT.Exp,
                                             scale=-1.0)
                        rden = stat_pool.tile([128, 2 * QT_PER_G], f32,
                                              tag="rden", name="rden")
                        nc.vector.reciprocal(rden[:], den_g[:])
                        scale_g = stat_pool.tile([128, 2 * QT_PER_G], f32,
                                                 tag="scale", name="scale_g")
                        nc.vector.tensor_tensor(scale_g[:], eneg[:], rden[:],
                                                op=ALU.mult)

                        # outputs: paired transposes, per-partition scale, DMA
                        for j in range(QT_PER_G):
                            qt_i = qg * QT_PER_G + j
                            jsl = slice(j * 128, (j + 1) * 128)
                            ps_trA = pst_pool.tile([128, 64], f32, tag="sT",
                                                   name="ps_trA")
                            nc.tensor.matmul(ps_trA[:], oT[0:64, jsl],
                                             ident[:], is_transpose=True,
                                             start=True, stop=True)
                            ps_trB = pst_pool.tile([128, 64], f32, tag="sT",
                                                   name="ps_trB")
                            nc.tensor.matmul(ps_trB[:], oT[64:128, jsl],
                                             identB[:], is_transpose=True,
                                             start=True, stop=True)
                            ofin = of_pool.tile([128, 128], f32, tag="ofin",
                                                name="ofin")
                            nc.scalar.activation(
                                ofin[:, 0:64], ps_trA[:], FT.Copy, bias=0.0,
                                scale=scale_g[:, 2 * j:2 * j + 1])
                            nc.scalar.activation(
                                ofin[:, 64:128], ps_trB[:], FT.Copy, bias=0.0,
                                scale=scale_g[:, 2 * j + 1:2 * j + 2])
                            nc.sync.dma_start(
                                out_d[qt_i * 128:(qt_i + 1) * 128,
                                      hp * 128:(hp + 1) * 128], ofin[:])

    nc.compile()
    return nc


def _shard_inputs(inputs):
    f32 = np.float32
    hs = np.asarray(inputs["hidden_states"], dtype=f32)
    Wq = np.asarray(inputs["Wq"], dtype=f32)
    Wk = np.asarray(inputs["Wk"], dtype=f32)
    Wv = np.asarray(inputs["Wv"], dtype=f32)
    bq = np.asarray(inputs["bq"], dtype=f32)
    bk = np.asarray(inputs["bk"], dtype=f32)
    bv = np.asarray(inputs["bv"], dtype=f32)
    in_maps = []
    for c in range(NCORES):
        b, hg = c % B, c // B
        sl = slice(hg * DG, (hg + 1) * DG)
        in_maps.append({
            "xt": np.ascontiguousarray(hs[b].T),
            "wqt": np.ascontiguousarray(Wq[sl].T),
            "wkt": np.ascontiguousarray(Wk[sl].T),
            "wvt": np.ascontiguousarray(Wv[sl].T),
            "bq8": np.ascontiguousarray((bq[sl] / 8.0).reshape(4, 128).T),
            "bk4": np.ascontiguousarray(bk[sl].reshape(4, 128).T),
            "bvrow": np.ascontiguousarray(bv[sl].reshape(1, DG)),
        })
    return in_maps


def kernel(**inputs):
    from concourse.bass_utils import run_bass_kernel_spmd

    if "nc" not in _CACHE:
        _CACHE["nc"] = build()
    nc = _CACHE["nc"]
    in_maps = _shard_inputs(inputs)
    res = run_bass_kernel_spmd(nc, in_maps, core_ids=list(range(NCORES)))
    out = np.empty((B, S, D), np.float32)
    for c in range(NCORES):
        b, hg = c % B, c // B
        out[b, :, hg * DG:(hg + 1) * DG] = res.results[c]["out"]
    return out


# revision 22
# speedup vs baseline: 1.1817x; 1.1817x over previous
"""Trainium2 Bass kernel for the buggy-softmax attention layer.

Shapes: hidden_states [4, 2048, 1024], Wq/Wk/Wv [1024, 1024], biases [1024].
reference: q,k,v proj -> per-head scores/sqrt(64) -> exp(s - max) / sum(RAW s)
-> @ v -> merge heads.

Sharding: 8 cores = 4 batches x 2 head-groups (8 heads of 64 dims each).
Each core gets x[b].T plus the head-group slice of the transposed weights and
produces out[b][:, hg*512:(hg+1)*512] in natural [seq, feat] layout.

Per-core algorithm (dual-orientation scores):
  qT = (x@Wq.T)/8 + bq/8   [512f, S]  fp32  (1/8 folded into the PSUM evac)
  kT = x@Wk.T + bk         [512f, S]  f32r  (rounded copy for fast matmuls)
  v  = x@Wv.T + bv         [S, 512f]  f32r  (bias via K=1 ones-matmul)
  den (sum of RAW scores) stays fp32-faithful: 128-key chunk sums of k
  (fp32, from the k PSUM) -> den = matmul(qT_h tile, ksum chunks) + DVE sum.
  m = rowmax(s) via f32r scores in [q,k] orientation + DVE reduce_max.
  numerator: sT = kT-tile^T contraction against qT -> exp(sT) -> E0T (sbuf);
  outT[64, q] += v-tile^T-contract @ E0T (f32r).
  out = transpose(outT) * exp(-m)/den (per-partition scale), DMA out.
"""

import numpy as np

B, S, D, H = 4, 2048, 1024, 16
HD = 64
NCORES = 8
HPC = 8            # heads per core
DG = HPC * HD      # 512 features per core

_CACHE = {}


def build(nc_S=S, dbg=False):
    import concourse.mybir as mybir
    import concourse.tile as tile
    from concourse import bacc
    from concourse.masks import make_identity

    f32 = mybir.dt.float32
    f32r = mybir.dt.float32r
    f16 = mybir.dt.float16
    FT = mybir.ActivationFunctionType
    ALU = mybir.AluOpType
    AX = mybir.AxisListType

    S_ = nc_S
    KP = D // 128            # 8 contraction chunks for projections
    PT = DG // 128           # 4 feature partition-tiles
    NW = min(512, S_)        # seq chunk width for projections / smax
    N5 = S_ // NW            # chunks per row
    ST = S_ // 128           # seq/key tiles of 128
    KC = S_ // 128           # 128-key ksum chunks
    QGW = min(512, S_)       # query-group width
    NQG = S_ // QGW
    QT_PER_G = QGW // 128

    nc = bacc.Bacc(None, target_bir_lowering=False, debug=dbg)

    xt_d = nc.declare_dram_parameter("xt", [D, S_], f32, isOutput=False)
    wqt_d = nc.declare_dram_parameter("wqt", [D, DG], f32, isOutput=False)
    wkt_d = nc.declare_dram_parameter("wkt", [D, DG], f32, isOutput=False)
    wvt_d = nc.declare_dram_parameter("wvt", [D, DG], f32, isOutput=False)
    bq8_d = nc.declare_dram_parameter("bq8", [128, PT], f32, isOutput=False)
    bk4_d = nc.declare_dram_parameter("bk4", [128, PT], f32, isOutput=False)
    bv_d = nc.declare_dram_parameter("bvrow", [1, DG], f32, isOutput=False)
    out_d = nc.declare_dram_parameter("out", [S_, DG], f32, isOutput=True)

    with tile.TileContext(nc) as tc:
        with (
            tc.tile_pool(name="const", bufs=1) as cpool,
            tc.tile_pool(name="qt", bufs=PT) as qt_pool,
            tc.tile_pool(name="ktr", bufs=PT) as ktr_pool,
            tc.tile_pool(name="vv", bufs=ST) as v_pool,
            tc.tile_pool(name="ksumc", bufs=PT) as ks_pool,
        ):
            ident = cpool.tile([64, 64], f32, tag="ident")
            make_identity(nc, ident[:])
            identB_t = cpool.tile([128, 64], f32, tag="identB")
            make_identity(nc, identB_t[64:128, :])
            identB = identB_t[64:128, :]
            bq8 = cpool.tile([128, PT], f32, tag="bq8")
            bk4 = cpool.tile([128, PT], f32, tag="bk4")
            bvb = cpool.tile([128, DG], f32, tag="bvb")
            nc.sync.dma_start(bq8[:], bq8_d[:])
            nc.sync.dma_start(bk4[:], bk4_d[:])
            nc.sync.dma_start(bvb[:], bv_d[:].to_broadcast([128, DG]))

            qt_sb = [qt_pool.tile([128, S_], f32, tag="qt", name="qt_sb") for _ in range(PT)]
            ktr_sb = [ktr_pool.tile([128, S_], f16, tag="ktr", name="ktr_sb") for _ in range(PT)]
            v_sb = [v_pool.tile([128, DG], f16, tag="vv", name="v_sb") for _ in range(ST)]
            ksumc = [ks_pool.tile([128, KC], f32, tag="ks", name="ksumc") for _ in range(PT)]

            # ---------------- Phase 1: projections ----------------
            with (
                tc.tile_pool(name="xt", bufs=KP) as xt_pool,
                tc.tile_pool(name="w", bufs=10) as w_pool,
                tc.tile_pool(name="xtr", bufs=3) as xtr_pool,
                tc.tile_pool(name="kss", bufs=2) as kss_pool,
                tc.tile_pool(name="p1", bufs=4, space="PSUM") as p1_pool,
            ):
                xt_sb = [xt_pool.tile([128, S_], f32, tag="xt", name="xt_sb")
                         for _ in range(KP)]
                for k8 in range(KP):
                    nc.sync.dma_start(xt_sb[k8][:],
                                      xt_d[k8 * 128:(k8 + 1) * 128, :])

                # --- q projection (fp32; 1/8 + bias folded into evac)
                wq = [w_pool.tile([128, DG], f32, tag="w", name="w_t") for _ in range(KP)]
                for k8 in range(KP):
                    nc.sync.dma_start(wq[k8][:],
                                      wqt_d[k8 * 128:(k8 + 1) * 128, :])
                for p in range(PT):
                    for n in range(N5):
                        ps = p1_pool.tile([128, NW], f32, tag="p1")
                        for k8 in range(KP):
                            nc.tensor.matmul(
                                ps[:], wq[k8][:, p * 128:(p + 1) * 128],
                                xt_sb[k8][:, n * NW:(n + 1) * NW],
                                start=(k8 == 0), stop=(k8 == KP - 1))
                        nc.scalar.activation(
                            qt_sb[p][:, n * NW:(n + 1) * NW], ps[:],
                            FT.Identity, bias=bq8[:, p:p + 1], scale=0.125)

                # --- k projection (fp32; f32r evac + fp32 128-key chunk sums)
                wk = [w_pool.tile([128, DG], f32, tag="w", name="w_t") for _ in range(KP)]
                for k8 in range(KP):
                    nc.sync.dma_start(wk[k8][:],
                                      wkt_d[k8 * 128:(k8 + 1) * 128, :])
                for p in range(PT):
                    for n in range(N5):
                        ps = p1_pool.tile([128, NW], f32, tag="p1")
                        for k8 in range(KP):
                            nc.tensor.matmul(
                                ps[:], wk[k8][:, p * 128:(p + 1) * 128],
                                xt_sb[k8][:, n * NW:(n + 1) * NW],
                                start=(k8 == 0), stop=(k8 == KP - 1))
                        nc.scalar.activation(
                            ktr_sb[p][:, n * NW:(n + 1) * NW], ps[:],
                            FT.Identity, bias=bk4[:, p:p + 1], scale=1.0)
                        nsub = NW // 128
                        for c in range(nsub):
                            ks_scr = kss_pool.tile([128, 128], f32, tag="kss")
                            col = n * nsub + c
                            nc.vector.tensor_scalar(
                                ks_scr[:], ps[:, c * 128:(c + 1) * 128],
                                bk4[:, p:p + 1], None, ALU.add,
                                op1=ALU.add,
                                accum_out=ksumc[p][:, col:col + 1])

                # --- v projection (f32r; bias via K=1 ones matmul)
                wv = [w_pool.tile([128, DG], f16, tag="w", name="wv_t") for _ in range(KP)]
                for k8 in range(KP):
                    nc.gpsimd.dma_start(wv[k8][:],
                                        wvt_d[k8 * 128:(k8 + 1) * 128, :])
                for st in range(ST):
                    ps = p1_pool.tile([128, DG], f32, tag="p1")
                    for k8 in range(KP):
                        xtr = xtr_pool.tile([128, 128], f16, tag="xtr")
                        nc.vector.tensor_copy(
                            xtr[:], xt_sb[k8][:, st * 128:(st + 1) * 128])
                        nc.tensor.matmul(ps[:], xtr[:], wv[k8][:],
                                         start=(k8 == 0), stop=(k8 == KP - 1))
                    nc.vector.tensor_tensor(
                        v_sb[st][:], ps[:], bvb[:], op=ALU.add)

            # ---------------- Phase 2: attention ----------------
            with (
                tc.tile_pool(name="qtr", bufs=2) as qtr_pool,
                tc.tile_pool(name="e0t", bufs=17) as e_pool,
                tc.tile_pool(name="outT", bufs=5) as ot_pool,
                tc.tile_pool(name="ofin", bufs=4) as of_pool,
                tc.tile_pool(name="stats", bufs=2) as stat_pool,
                tc.tile_pool(name="pst", bufs=4, space="PSUM") as pst_pool,
                tc.tile_pool(name="psmax", bufs=3, space="PSUM") as psx_pool,
                tc.tile_pool(name="psav", bufs=1, space="PSUM") as pav_pool,
            ):
                for hp in range(HPC // 2):
                    h0, h1 = 2 * hp, 2 * hp + 1
                    qt_p = qt_sb[hp]                      # [128,S] fp32
                    kt_p = ktr_sb[hp]                     # [128,S] f32r
                    ks_p = ksumc[hp]                      # [128,KC] fp32

                    qtr_t = qtr_pool.tile([128, S_], f16, tag="qtr",
                                          name="qtr_t")
                    nc.vector.tensor_copy(qtr_t[:], qt_p[:])

                    for qg in range(NQG):
                        qsl = slice(qg * QGW, (qg + 1) * QGW)
                        # max + den per query tile (row-packed head pairs)
                        m2 = stat_pool.tile([128, 8 * QT_PER_G], f32,
                                            tag="m2", name="m2")
                        den_g = stat_pool.tile([128, 2 * QT_PER_G], f32,
                                               tag="den", name="den_g")
                        NKH = S_ // NW
                        for j in range(QT_PER_G):
                            qt_i = qg * QT_PER_G + j
                            qq = slice(qt_i * 128, (qt_i + 1) * 128)
                            for kh in range(NKH):
                                c0 = kh * NW
                                psxA = psx_pool.tile([128, NW], f32,
                                                     tag="smax", name="psxA")
                                nc.tensor.matmul(
                                    psxA[:], qtr_t[0:64, qq],
                                    kt_p[0:64, c0:c0 + NW],
                                    start=True, stop=True)
                                psxB = psx_pool.tile([128, NW], f32,
                                                     tag="smax", name="psxB")
                                nc.tensor.matmul(
                                    psxB[:], qtr_t[64:128, qq],
                                    kt_p[64:128, c0:c0 + NW],
                                    start=True, stop=True)
                                nc.vector.reduce_max(
                                    m2[:, 8 * j + kh:8 * j + kh + 1],
                                    psxA[:], axis=AX.X)
                                nc.vector.reduce_max(
                                    m2[:, 8 * j + 4 + kh:8 * j + 4 + kh + 1],
                                    psxB[:], axis=AX.X)
                            ps_dA = pst_pool.tile([128, KC], f32, tag="sT",
                                                  name="ps_dA")
                            nc.tensor.matmul(ps_dA[:], qt_p[0:64, qq],
                                             ks_p[0:64, :],
                                             start=True, stop=True)
                            nc.vector.reduce_sum(
                                den_g[:, 2 * j:2 * j + 1],
                                ps_dA[:], axis=AX.X)
                            ps_dB = pst_pool.tile([128, KC], f32, tag="sT",
                                                  name="ps_dB")
                            nc.tensor.matmul(ps_dB[:], qt_p[64:128, qq],
                                             ks_p[64:128, :],
                                             start=True, stop=True)
                            nc.vector.reduce_sum(
                                den_g[:, 2 * j + 1:2 * j + 2],
                                ps_dB[:], axis=AX.X)

                        # numerator: paired sT tiles -> exp -> E0T; then AV
                        e0, e1 = [], []
                        for kt in range(ST):
                            ksl = slice(kt * 128, (kt + 1) * 128)
                            psA = pst_pool.tile([128, QGW], f32, tag="sT",
                                                name="psA")
                            nc.tensor.matmul(psA[:], kt_p[0:64, ksl],
                                             qtr_t[0:64, qsl],
                                             start=True, stop=True)
                            psB = pst_pool.tile([128, QGW], f32, tag="sT",
                                                name="psB")
                            nc.tensor.matmul(psB[:], kt_p[64:128, ksl],
                                             qtr_t[64:128, qsl],
                                             start=True, stop=True)
                            etA = e_pool.tile([128, QGW], f16, tag="e0t",
                                              name="etA")
                            nc.scalar.activation(etA[:], psA[:], FT.Exp)
                            etB = e_pool.tile([128, QGW], f16, tag="e0t",
                                              name="etB")
                            nc.scalar.activation(etB[:], psB[:], FT.Exp)
                            e0.append(etA)
                            e1.append(etB)
                        ps_av = pav_pool.tile([64, QGW], f32, tag="av",
                                              name="ps_av")
                        for kt in range(ST):
                            nc.tensor.matmul(
                                ps_av[:], v_sb[kt][:, h0 * 64:h0 * 64 + 64],
                                e0[kt][:],
                                start=(kt == 0), stop=(kt == ST - 1))
                        ps_av2 = pav_pool.tile([64, QGW], f32, tag="av",
                                               name="ps_av2")
                        for kt in range(ST):
                            nc.tensor.matmul(
                                ps_av2[:], v_sb[kt][:, h1 * 64:h1 * 64 + 64],
                                e1[kt][:],
                                start=(kt == 0), stop=(kt == ST - 1))
                        oT = ot_pool.tile([128, QGW], f32, tag="outT",
                                          name="oT")
                        nc.scalar.copy(oT[0:64, :], ps_av[:])
                        nc.scalar.copy(oT[64:128, :], ps_av2[:])

                        # per-qg stats: scale = exp(-m) / den
                        m_g = stat_pool.tile([128, 2 * QT_PER_G], f32,
                                             tag="m", name="m_g")
                        if NKH == 4:
                            nc.vector.reduce_max(
                                m_g[:],
                                m2[:].rearrange("p (q x) -> p q x", x=4),
                                axis=AX.X)
                        else:
                            nc.vector.reduce_max(
                                m_g[:],
                                m2[:].rearrange("p (q x) -> p q x",
                                                x=4)[:, :, 0:NKH],
                                axis=AX.X)
                        eneg = stat_pool.tile([128, 2 * QT_PER_G], f32,
                                              tag="eneg", name="eneg")
                        nc.scalar.activation(eneg[:], m_g[:], FT.Exp,
                                             scale=-1.0)
                        rden = stat_pool.tile([128, 2 * QT_PER_G], f32,
                                              tag="rden", name="rden")
                        nc.vector.reciprocal(rden[:], den_g[:])
                        scale_g = stat_pool.tile([128, 2 * QT_PER_G], f32,
                                                 tag="scale", name="scale_g")
                        nc.vector.tensor_tensor(scale_g[:], eneg[:], rden[:],
                                                op=ALU.mult)

                        # outputs: paired transposes, per-partition scale, DMA
                        for j in range(QT_PER_G):
                            qt_i = qg * QT_PER_G + j
                            jsl = slice(j * 128, (j + 1) * 128)
                            ps_trA = pst_pool.tile([128, 64], f32, tag="sT",
                                                   name="ps_trA")
                            nc.tensor.matmul(ps_trA[:], oT[0:64, jsl],
                                             ident[:], is_transpose=True,
                                             start=True, stop=True)
                            ps_trB = pst_pool.tile([128, 64], f32, tag="sT",
                                                   name="ps_trB")
                            nc.tensor.matmul(ps_trB[:], oT[64:128, jsl],
                                             identB[:], is_transpose=True,
                                             start=True, stop=True)
                            ofin = of_pool.tile([128, 128], f32, tag="ofin",
                                                name="ofin")
                            nc.scalar.activation(
                                ofin[:, 0:64], ps_trA[:], FT.Copy, bias=0.0,
                                scale=scale_g[:, 2 * j:2 * j + 1])
                            nc.scalar.activation(
                                ofin[:, 64:128], ps_trB[:], FT.Copy, bias=0.0,
                                scale=scale_g[:, 2 * j + 1:2 * j + 2])
                            nc.sync.dma_start(
                                out_d[qt_i * 128:(qt_i + 1) * 128,
                                      hp * 128:(hp + 1) * 128], ofin[:])

    nc.compile()
    return nc


def _shard_inputs(inputs):
    f32 = np.float32
    hs = np.asarray(inputs["hidden_states"], dtype=f32)
    Wq = np.asarray(inputs["Wq"], dtype=f32)
    Wk = np.asarray(inputs["Wk"], dtype=f32)
    Wv = np.asarray(inputs["Wv"], dtype=f32)
    bq = np.asarray(inputs["bq"], dtype=f32)
    bk = np.asarray(inputs["bk"], dtype=f32)
    bv = np.asarray(inputs["bv"], dtype=f32)
    in_maps = []
    for c in range(NCORES):
        b, hg = c % B, c // B
        sl = slice(hg * DG, (hg + 1) * DG)
        in_maps.append({
            "xt": np.ascontiguousarray(hs[b].T),
            "wqt": np.ascontiguousarray(Wq[sl].T),
            "wkt": np.ascontiguousarray(Wk[sl].T),
            "wvt": np.ascontiguousarray(Wv[sl].T),
            "bq8": np.ascontiguousarray((bq[sl] / 8.0).reshape(4, 128).T),
            "bk4": np.ascontiguousarray(bk[sl].reshape(4, 128).T),
            "bvrow": np.ascontiguousarray(bv[sl].reshape(1, DG)),
        })
    return in_maps


def kernel(**inputs):
    import os
    # The NTFF trace hook isn't available outside the dev container; make
    # sure run_bass_kernel_spmd never takes the tracing path here.
    os.environ["BASS_NEVER_TRACE"] = "1"
    from concourse.bass_utils import run_bass_kernel_spmd

    if "nc" not in _CACHE:
        _CACHE["nc"] = build()
    nc = _CACHE["nc"]
    in_maps = _shard_inputs(inputs)
    res = run_bass_kernel_spmd(nc, in_maps, core_ids=list(range(NCORES)))
    out = np.empty((B, S, D), np.float32)
    for c in range(NCORES):
        b, hg = c % B, c // B
        out[b, :, hg * DG:(hg + 1) * DG] = res.results[c]["out"]
    return out
